# revision 1
# baseline (speedup 1.0000x reference)
"""Causal self-attention (B=2, T=2048, C=1024, H=16) on 8 trn2 NeuronCores.

Sharding: core = (batch b, head-group g). 2 batches x 4 head-groups of 4
heads. Each core computes QKV for its head shard, full causal attention for
its 4 heads, and a partial output projection (row-parallel W_proj). Host sums
the 4 partials per batch (the row-parallel "all-reduce") and transposes.

Per-core device kernel (fp32r matmul operands, expS/V in bf16):
  phase 1: QK^T [512,2048] = Wqk^T @ x^T   (weights stay in natural layout)
           V   [2048,256+ones] = x @ Wv    (natural layout for 2nd matmul)
  phase 2: heads processed in pairs (head 2p at array rows 0-63, head 2p+1 at
           rows 64-127 -> concurrent K=64 matmuls). Per j-tile (128 keys) and
           512-col i-chunk: S^T for both heads into one [128,1024] PSUM tile,
           one Exp (fused 1/8 scale) into a pair-interleaved bf16 expS stage,
           causal mask on the diagonal chunk only. Then U^T[65,512] =
           V'_h^T @ expS accumulated over j-tiles -- row 64 is the softmax
           denominator (ones column in V'). Normalize with DVE reciprocal +
           rank-1 ones matmul broadcast.
  phase 3: outT_partial [1024,2048] = Wp_shard^T @ Y^T + b_proj/4,
           interleaved into pass B of the second pair.
"""

import numpy as np
import ml_dtypes

B, T, C, H = 2, 2048, 1024, 16
D = C // H          # 64
HL = 4              # heads per core
DL = HL * D         # 256 local head dim
NCORES = 8

# expS row offsets: row jb covers i in [512*(jb//4), 2048); pair-interleaved
# layout holds (h0 512 | h1 512) per 512-wide i-chunk.
_LENS = [2048 - 512 * (jb // 4) for jb in range(16)]
_OFF = [0] * 16
for _jb in range(1, 16):
    _OFF[_jb] = _OFF[_jb - 1] + _LENS[_jb - 1]
_ETOT = _OFF[15] + _LENS[15]  # 20480


def _build_nc():
    from contextlib import ExitStack
    import concourse.bacc as bacc
    import concourse.tile as tile
    from concourse import mybir

    f32 = mybir.dt.float32
    f32r = mybir.dt.float32r
    bf16 = mybir.dt.bfloat16
    AF = mybir.ActivationFunctionType

    nc = bacc.Bacc("TRN2", target_bir_lowering=False, debug=False)

    xT = nc.dram_tensor("xT", [C, T], bf16, kind="ExternalInput")
    wqk_d = nc.dram_tensor("wqk", [C, 2 * DL], bf16, kind="ExternalInput")
    wv_d = nc.dram_tensor("wv", [C, DL], bf16, kind="ExternalInput")
    bqk_d = nc.dram_tensor("bqk", [128, 4], f32, kind="ExternalInput")
    bvb_d = nc.dram_tensor("bvb", [128, DL], f32, kind="ExternalInput")
    wp_d = nc.dram_tensor("wp", [DL, C], f32r, kind="ExternalInput")
    bp_d = nc.dram_tensor("bp", [128, 8], f32, kind="ExternalInput")
    mk_d = nc.dram_tensor("masks", [128, 4, 1024], bf16, kind="ExternalInput")
    ones_d = nc.dram_tensor("ones64", [1, 64], f32r, kind="ExternalInput")
    outT = nc.dram_tensor("outT", [C, T], f32, kind="ExternalOutput")

    with tile.TileContext(nc) as tc, ExitStack() as ctx:
        const = ctx.enter_context(tc.tile_pool(name="const", bufs=1))
        qkT_pool = ctx.enter_context(tc.tile_pool(name="qkT", bufs=1))
        vp_pool = ctx.enter_context(tc.tile_pool(name="vp", bufs=1))
        yt_pool = ctx.enter_context(tc.tile_pool(name="yt", bufs=1))
        eA_pool = ctx.enter_context(tc.tile_pool(name="eA", bufs=1))
        ps_s = ctx.enter_context(tc.tile_pool(name="ps_s", bufs=2, space="PSUM"))

        bqk_sb = const.tile([128, 4], f32, tag="bqk", name="bqk")
        nc.sync.dma_start(out=bqk_sb, in_=bqk_d[:])

        qkT = [qkT_pool.tile([128, T], bf16, tag=f"qkT{m}", name=f"qkT{m}")
               for m in range(4)]
        Vp = vp_pool.tile([128, 16, HL, 66], bf16, tag="vp", name="vp")
        YT = [yt_pool.tile([128, T], f32r, tag=f"yt{k}", name=f"yt{k}")
              for k in range(2)]

        # ones column of V' (softmax denominator trick); two bf16 1.0s per
        # f32 slot (DVE memset only takes f32 value types)
        nc.vector.memset(Vp[:, :, :, 64:66].bitcast(f32),
                         float(np.uint32(0x3F803F80).view(np.float32)))

        # expS staging, pair-interleaved (h0 512 | h1 512) per 512-col
        # i-chunk; one full-pair slot, pairs rotate via WAR deps.
        es_a = {0: eA_pool.tile([128, 2 * _ETOT], bf16, tag="eA", name="eA")}

        def loc(p, jb):
            return es_a[p], 2 * _OFF[jb]

        def s_chunk(p, jb, ic):
            eS_t, off = loc(p, jb)
            i0 = 512 * (jb // 4)
            QT, KT = qkT[p], qkT[2 + p]
            ps = ps_s.tile([128, 1024], f32, tag="ps_s", name="ps_s")
            for hh in range(2):
                nc.tensor.matmul(
                    ps[:, 512 * hh:512 * (hh + 1)],
                    KT[64 * hh:64 * hh + 64, 128 * jb:128 * (jb + 1)],
                    QT[64 * hh:64 * hh + 64,
                       i0 + 512 * ic:i0 + 512 * (ic + 1)],
                    start=True, stop=True)
            nc.scalar.activation(
                out=eS_t[:, off + 1024 * ic:off + 1024 * (ic + 1)],
                in_=ps, func=AF.Exp, scale=0.125)

        def mask_row(p, jb):
            eS_t, off = loc(p, jb)
            nc.vector.tensor_mul(out=eS_t[:, off:off + 1024],
                                 in0=eS_t[:, off:off + 1024],
                                 in1=mk_sb[:, jb % 4, :])

        # ---------------- phase 1: QK^T, V, early S rows ----------------
        with ExitStack() as p1:
            w_pool = p1.enter_context(tc.tile_pool(name="w1", bufs=1))
            xb_pool = p1.enter_context(tc.tile_pool(name="xb", bufs=2))
            ps_qk = p1.enter_context(tc.tile_pool(name="ps_qk", bufs=2, space="PSUM"))
            ps_v = p1.enter_context(tc.tile_pool(name="ps_v", bufs=2, space="PSUM"))

            # DMA order: interleave W columns with the first x block so the
            # first matmul chain starts ASAP; wv/masks later, wp/ones last.
            wqk_sb, wv_sb = [], []
            xb0 = []
            for c in range(8):
                t_ = w_pool.tile([128, 2 * DL], bf16, tag=f"wqk{c}", name=f"wqk{c}")
                nc.sync.dma_start(out=t_, in_=wqk_d[128 * c:128 * (c + 1), :])
                wqk_sb.append(t_)
                t_ = xb_pool.tile([128, 512], bf16, tag=f"xb{c}", name=f"xb{c}")
                nc.sync.dma_start(out=t_, in_=xT[128 * c:128 * (c + 1), 0:512])
                xb0.append(t_)
            mk_sb = const.tile([128, 4, 1024], bf16, tag="mk", name="mk")
            nc.sync.dma_start(out=mk_sb, in_=mk_d[:])
            for c in range(8):
                t_ = w_pool.tile([128, DL], bf16, tag=f"wv{c}", name=f"wv{c}")
                nc.sync.dma_start(out=t_, in_=wv_d[128 * c:128 * (c + 1), :])
                wv_sb.append(t_)
            bvb_sb = const.tile([128, DL], f32, tag="bvb", name="bvb")
            nc.sync.dma_start(out=bvb_sb, in_=bvb_d[:])

            def qk_part(m, i4, xb):
                ps = ps_qk.tile([128, 512], f32, tag="ps_qk", name="ps_qk")
                for c in range(8):
                    nc.tensor.matmul(
                        ps, wqk_sb[c][:, 128 * m:128 * (m + 1)], xb[c],
                        start=(c == 0), stop=(c == 7))
                nc.vector.tensor_scalar_add(
                    out=qkT[m][:, 512 * i4:512 * (i4 + 1)], in0=ps,
                    scalar1=bqk_sb[:, m:m + 1])

            for i4 in range(4):
                if i4 == 0:
                    xb = xb0
                else:
                    xb = []
                    for c in range(8):
                        t_ = xb_pool.tile([128, 512], bf16, tag=f"xb{c}",
                                          name=f"xb{c}")
                        nc.sync.dma_start(
                            out=t_,
                            in_=xT[128 * c:128 * (c + 1),
                                   512 * i4:512 * (i4 + 1)])
                        xb.append(t_)
                qk_part(0, i4, xb)
                qk_part(2, i4, xb)
                # early S for pair 0: row block jbb (K cols ready at
                # i4 >= jbb) gets its chunk (i-cols 512*i4..) as Q lands
                for jbb in range(i4 + 1):
                    for jb in range(4 * jbb, 4 * jbb + 4):
                        s_chunk(0, jb, i4 - jbb)
                        if i4 == jbb:
                            mask_row(0, jb)
                qk_part(1, i4, xb)
                qk_part(3, i4, xb)
                for i1 in range(4):
                    jb = 4 * i4 + i1
                    psv = ps_v.tile([128, DL], f32, tag="ps_v", name="ps_v")
                    for c in range(8):
                        nc.tensor.matmul(
                            psv, xb[c][:, 128 * i1:128 * (i1 + 1)], wv_sb[c],
                            start=(c == 0), stop=(c == 7))
                    nc.vector.tensor_add(
                        out=Vp[:, jb, :, 0:64],
                        in0=psv.rearrange("p (h d) -> p h d", h=HL),
                        in1=bvb_sb.rearrange("p (h d) -> p h d", h=HL))

        # weights for later phases (queued behind phase-1-critical DMAs)
        wp_sb = []
        for kt in range(2):
            t_ = const.tile([128, C], f32r, tag=f"wp{kt}", name=f"wp{kt}")
            nc.sync.dma_start(out=t_, in_=wp_d[128 * kt:128 * (kt + 1), :])
            wp_sb.append(t_)
        ones64 = const.tile([1, 64], f32r, tag="ones64", name="ones64")
        nc.sync.dma_start(out=ones64, in_=ones_d[:])
        bp_sb = const.tile([128, 8], f32, tag="bp", name="bp")
        nc.sync.dma_start(out=bp_sb, in_=bp_d[:])

        # ---------------- phase 2 + 3: attention, projection ----------------
        with ExitStack() as p2:
            rin_pool = p2.enter_context(tc.tile_pool(name="rin", bufs=6))
            ytmp_pool = p2.enter_context(tc.tile_pool(name="ytmp", bufs=3))
            bc_pool = p2.enter_context(tc.tile_pool(name="bc", bufs=4))
            ost_pool = p2.enter_context(tc.tile_pool(name="ost", bufs=6))
            ps_u = p2.enter_context(tc.tile_pool(name="ps_u", bufs=2, space="PSUM"))
            ps_b = p2.enter_context(tc.tile_pool(name="ps_b", bufs=1, space="PSUM"))
            ps_o = p2.enter_context(tc.tile_pool(name="ps_o", bufs=1, space="PSUM"))

            def emit_proj_m(it, m):
                pso = ps_o.tile([128, 512], f32, tag="ps_o", name="ps_o")
                for kt in range(2):
                    nc.tensor.matmul(
                        pso,
                        wp_sb[kt][:, 128 * m:128 * (m + 1)],
                        YT[kt][:, 512 * it:512 * (it + 1)],
                        start=(kt == 0), stop=(kt == 1))
                ot = ost_pool.tile([128, 512], f32, tag="ot", name="ot")
                nc.vector.tensor_scalar_add(
                    out=ot, in0=pso, scalar1=bp_sb[:, m:m + 1])
                nc.sync.dma_start(
                    out=outT[128 * m:128 * (m + 1), 512 * it:512 * (it + 1)],
                    in_=ot)

            # proj work for a completed it is interleaved into the next it's
            # U chains so the single ps_o buffer's DVE eviction hides behind
            # U matmuls instead of stalling the PE
            proj_queue = []

            for p in range(2):
                if p == 1:
                    es_a[1] = eA_pool.tile([128, 2 * _ETOT], bf16, tag="eA",
                                           name="eA")
                for it in range(4):
                    for jb in range(4 * it, 4 * it + 4):
                        if p == 0:
                            continue  # emitted during phase 1
                        for ic in range(_LENS[jb] // 512):
                            s_chunk(p, jb, ic)
                        mask_row(p, jb)
                    njb = 4 * it + 4
                    for hh in range(2):
                        h = 2 * p + hh
                        ups = ps_u.tile([65, 512], f32, tag="ps_u", name="ps_u")
                        for jb in range(njb):
                            eS_t, off = loc(p, jb)
                            o = off + 1024 * (it - jb // 4) + 512 * hh
                            nc.tensor.matmul(
                                ups, Vp[:, jb, h, 0:65], eS_t[:, o:o + 512],
                                start=(jb == 0), stop=(jb == njb - 1))
                            if proj_queue and jb % 2 == 1:
                                emit_proj_m(*proj_queue.pop(0))
                        rin = rin_pool.tile([1, 512], f32r, tag="rin",
                                            name="rin")
                        with nc.allow_low_precision(reason="f32r bcast"):
                            nc.vector.reciprocal(rin, ups[64:65, :])
                        bps = ps_b.tile([64, 512], f32, tag="ps_b", name="ps_b")
                        nc.tensor.matmul(bps, ones64, rin, start=True,
                                         stop=True)
                        bc_sb = bc_pool.tile([64, 512], f32, tag="bc",
                                             name="bc")
                        nc.vector.tensor_copy(out=bc_sb, in_=bps)
                        if hh == 0:
                            nc.vector.tensor_mul(
                                out=YT[p][0:64, 512 * it:512 * (it + 1)],
                                in0=ups[0:64, :], in1=bc_sb)
                        else:
                            yt2 = ytmp_pool.tile([64, 512], f32r, tag="yt2",
                                                 name="yt2")
                            nc.vector.tensor_mul(out=yt2, in0=ups[0:64, :],
                                                 in1=bc_sb)
                            nc.sync.dma_start(
                                out=YT[p][64:128, 512 * it:512 * (it + 1)],
                                in_=yt2)
                    if p == 1 and it < 3:
                        proj_queue.extend((it, m) for m in range(8))

        # tail: it=3 proj with a deep PSUM pool (pass-B banks are free now)
        with ExitStack() as p3:
            ps_o2 = p3.enter_context(tc.tile_pool(name="ps_o2", bufs=4,
                                                  space="PSUM"))
            ost2_pool = p3.enter_context(tc.tile_pool(name="ost2", bufs=4))
            for m in range(8):
                pso = ps_o2.tile([128, 512], f32, tag="ps_o2", name="ps_o2")
                for kt in range(2):
                    nc.tensor.matmul(
                        pso,
                        wp_sb[kt][:, 128 * m:128 * (m + 1)],
                        YT[kt][:, 1536:2048],
                        start=(kt == 0), stop=(kt == 1))
                ot = ost2_pool.tile([128, 512], f32, tag="ot2", name="ot2")
                nc.vector.tensor_scalar_add(
                    out=ot, in0=pso, scalar1=bp_sb[:, m:m + 1])
                nc.sync.dma_start(
                    out=outT[128 * m:128 * (m + 1), 1536:2048], in_=ot)

    nc.compile()
    return nc


_NC = None


def _get_nc():
    global _NC
    if _NC is None:
        _NC = _build_nc()
    return _NC


def _masks_np():
    p = np.arange(128)[:, None, None]
    r = np.arange(4)[None, :, None]
    f = np.arange(512)[None, None, :]
    m = ((p + 128 * r) <= f).astype(ml_dtypes.bfloat16)
    return np.concatenate([m, m], axis=-1)  # duplicated for the head pair


def make_in_maps(x, W_attn, b_attn, W_proj, b_proj):
    x = np.asarray(x, np.float32)
    W_attn = np.asarray(W_attn, np.float32)
    b_attn = np.asarray(b_attn, np.float32)
    W_proj = np.asarray(W_proj, np.float32)
    b_proj = np.asarray(b_proj, np.float32)
    masks = _masks_np()
    in_maps = []
    for core in range(NCORES):
        b, g = divmod(core, 4)
        hs = slice(g * DL, (g + 1) * DL)
        wq = W_attn[:, 0:C][:, hs]
        wk = W_attn[:, C:2 * C][:, hs]
        wv = W_attn[:, 2 * C:3 * C][:, hs]
        bq = b_attn[0:C][hs]
        bk = b_attn[C:2 * C][hs]
        bv = b_attn[2 * C:3 * C][hs]
        in_maps.append({
            "xT": np.ascontiguousarray(x[b].T).astype(ml_dtypes.bfloat16),
            "wqk": np.ascontiguousarray(
                np.concatenate([wq, wk], axis=1)).astype(ml_dtypes.bfloat16),
            "wv": np.ascontiguousarray(wv).astype(ml_dtypes.bfloat16),
            "bqk": np.ascontiguousarray(
                np.concatenate([bq, bk]).reshape(4, 128).T),
            "bvb": np.broadcast_to(bv, (128, DL)).copy(),
            "wp": np.ascontiguousarray(W_proj[hs, :]),
            "bp": np.ascontiguousarray((b_proj * 0.25).reshape(8, 128).T),
            "masks": masks,
            "ones64": np.ones((1, 64), np.float32),
        })
    return in_maps


def assemble_output(results):
    out = np.empty((B, T, C), np.float32)
    for b in range(B):
        acc = results[4 * b]["outT"].astype(np.float32)
        for g in range(1, 4):
            acc = acc + results[4 * b + g]["outT"]
        out[b] = acc.T
    return out


def kernel(x, W_attn, b_attn, W_proj, b_proj):
    from concourse.bass_utils import run_bass_kernel_spmd
    nc = _get_nc()
    in_maps = make_in_maps(x, W_attn, b_attn, W_proj, b_proj)
    res = run_bass_kernel_spmd(nc, in_maps, core_ids=list(range(NCORES)))
    return assemble_output(res.results)

